# revision 2
# baseline (speedup 1.0000x reference)
"""Trainium2 Bass kernel for nn_BoundaryExtractionModule.

Data-parallel over batch: 8 samples -> 8 NeuronCores, one sample per core.

Per-core pipeline (channel-major layout [C, N] with C=64 on partitions):
  conv3x3(W_std)+depthwise-Laplacian   : 9 shift-matmuls per 512-col chunk
                                         (Laplacian folded into the taps on host)
  3-scale pooled non-local attention   : for each scale s in (4, 2, 1):
      A: row-max of logits  S = f^T f   (fp16 matmuls, DVE reduce_max)
      B: recompute S^T with the shift folded in via an augmented
         contraction row (K=65):  S'[m,q] = sum_k f_a[k,m] g_a[k,q]
         where f_a = [f; 1], g_a = [f; -rowmax]
      exp on ACT (PSUM -> fp16 SBUF)   : E^T tiles
      C: PV matmul with ones-column    : G = [f; 1] @ E^T  ->  G[64] = softmax denom
      D: out = G[0:64] * (1/G[64]) broadcast via a K=1 PE replication matmul
  bilinear x2/x4 upsample (half-pixel) : strided DVE ops on edge-padded buffers
  residual add + DMA out.
"""

import numpy as np

import concourse.bass as bass
import concourse.mybir as mybir
import concourse.tile as tile
from concourse import bacc
from concourse.bass_utils import run_bass_kernel_spmd
from concourse.masks import make_identity

dt = mybir.dt
AF = mybir.ActivationFunctionType
ALU = mybir.AluOpType
AX = mybir.AxisListType

C = 64
H = W = 64
N1 = H * W          # 4096
PAD = 66            # padded row length for conv
NCORES = 8

_cache = {}


def _v(ap, off, dims):
    """View of `ap` at free-offset `off` with free dims `dims` (keeps partition dim)."""
    return bass.AP(ap.tensor, ap.offset + off, [list(ap.ap[0])] + [list(d) for d in dims])


def _chunks(total, size):
    out = []
    off = 0
    while off < total:
        out.append((off, min(size, total - off)))
        off += size
    return out


def _build_nc():
    nc = bacc.Bacc(None, target_bir_lowering=False)
    xp_d = nc.dram_tensor("xp", [C, PAD * PAD], dt.float16, kind="ExternalInput")
    wt_d = nc.dram_tensor("wt", [C, 9 * C], dt.float16, kind="ExternalInput")
    out_d = nc.dram_tensor("out", [C, N1], dt.float32, kind="ExternalOutput")

    with tile.TileContext(nc) as tc:
        with (
            tc.tile_pool(name="sb", bufs=1) as sb,
            tc.tile_pool(name="ga", bufs=2) as ga_pool,
            tc.tile_pool(name="et", bufs=3) as et_pool,
            tc.tile_pool(name="dd", bufs=2) as dd_pool,
            tc.tile_pool(name="cm", bufs=2) as cm_pool,
            tc.tile_pool(name="pp", bufs=2, space="PSUM") as pp,
            tc.tile_pool(name="gg", bufs=1, space="PSUM") as gg,
            tc.tile_pool(name="rr", bufs=1, space="PSUM") as rr,
        ):
            # ---------------- inputs / constants ----------------
            xp16 = sb.tile([C, PAD * PAD], dt.float16)
            nc.sync.dma_start(xp16[:], xp_d.ap())
            wt16 = sb.tile([C, 9 * C], dt.float16)
            nc.sync.dma_start(wt16[:], wt_d.ap())

            ident = sb.tile([128, 128], dt.float16)
            make_identity(nc, ident[:])
            ones_rep = sb.tile([1, C], dt.float16)     # lhsT of the K=1 replication matmul
            nc.vector.memset(ones_rep[:], 1.0)

            out_acc = sb.tile([C, N1], dt.float32)
            # residual init: out_acc = x  (from the padded fp16 input)
            nc.vector.tensor_copy(out_acc[:], _v(xp16[:], PAD + 1, [[PAD, H], [1, W]]))

            # ---------------- conv (feat -> f1a) ----------------
            f1a = sb.tile([C + 1, N1], dt.float16)
            for r in range(8):
                cp = pp.tile([C, 512], dt.float32, tag="b")
                for tap in range(9):
                    dy, dx = divmod(tap, 3)
                    rhs = _v(xp16[:], (8 * r + dy) * PAD + dx, [[PAD, 8], [1, W]])
                    nc.tensor.matmul(cp[:], wt16[:, tap * C:(tap + 1) * C], rhs,
                                     start=(tap == 0), stop=(tap == 8))
                nc.scalar.copy(f1a[0:C, r * 512:(r + 1) * 512], cp[:])
            nc.vector.memset(f1a[C:C + 1, :], 1.0)

            # ---------------- pools (f2a, f4a) ----------------
            f1 = f1a[0:C, :]
            t2w = sb.tile([C, 2048], dt.float32)
            nc.vector.tensor_tensor(t2w[:], _v(f1, 0, [[2, 2048]]), _v(f1, 1, [[2, 2048]]), op=ALU.add)
            f2raw = sb.tile([C, 1024], dt.float32)
            nc.vector.tensor_tensor(f2raw[:], _v(t2w[:], 0, [[64, 32], [1, 32]]),
                                    _v(t2w[:], 32, [[64, 32], [1, 32]]), op=ALU.add)
            f2a = sb.tile([C + 1, 1024], dt.float16)
            nc.vector.tensor_scalar_mul(f2a[0:C, :], f2raw[:], 0.25)
            nc.vector.memset(f2a[C:C + 1, :], 1.0)

            t4w = sb.tile([C, 512], dt.float32)
            nc.vector.tensor_tensor(t4w[:], _v(f2raw[:], 0, [[2, 512]]), _v(f2raw[:], 1, [[2, 512]]), op=ALU.add)
            f4raw = sb.tile([C, 256], dt.float32)
            nc.vector.tensor_tensor(f4raw[:], _v(t4w[:], 0, [[32, 16], [1, 16]]),
                                    _v(t4w[:], 16, [[32, 16], [1, 16]]), op=ALU.add)
            f4a = sb.tile([C + 1, 256], dt.float16)
            nc.vector.tensor_scalar_mul(f4a[0:C, :], f4raw[:], 1.0 / 16.0)
            nc.vector.memset(f4a[C:C + 1, :], 1.0)

            att2p = sb.tile([C, 34 * 34], dt.float32)   # scale-2 attn out, 1-px padded
            att4p = sb.tile([C, 18 * 18], dt.float32)   # scale-4 attn out, 1-px padded

            # ---------------- generic attention ----------------
            def build_fT(fa, NT, name):
                fT = sb.tile([128, NT * 65], dt.float16, tag=name)
                nc.vector.memset(_v(fT[:], C, [[65, NT]]), 1.0)
                for j in range(NT):
                    pt = pp.tile([128, C], dt.float16, tag="b")
                    nc.tensor.transpose(pt[:], fa[0:C, j * 128:(j + 1) * 128], ident[0:C, 0:C])
                    nc.vector.tensor_copy(fT[:, j * 65:j * 65 + C], pt[:])
                return fT

            def attn(fa, fT, N, write_out):
                NT = N // 128
                for isb, (q0, Q) in enumerate(_chunks(N, 512)):
                    nsub = Q // 128
                    ga = ga_pool.tile([C + 1, Q], dt.float16, tag="ga")
                    nc.vector.tensor_copy(ga[0:C, :], fa[0:C, q0:q0 + Q])
                    # --- A: row maxes ---
                    for sub in range(nsub):
                        lhsA = fa[0:C, q0 + sub * 128: q0 + (sub + 1) * 128]
                        achunks = _chunks(N, 1536)
                        x2 = cm_pool.tile([128, 32], dt.float16, tag="x2")
                        if len(achunks) > 1:
                            x1 = cm_pool.tile([128, 4], dt.float32, tag="x1")
                        for k, (off, ln) in enumerate(achunks):
                            at = pp.tile([128, ln], dt.float32, tag="b")
                            for h0, hl in _chunks(ln, 512):
                                nc.tensor.matmul(at[:, h0:h0 + hl], lhsA,
                                                 fa[0:C, off + h0:off + h0 + hl],
                                                 start=True, stop=True)
                            if len(achunks) == 1:
                                nc.vector.reduce_max(x2[:, 0:1], at[:], axis=AX.X, negate=True)
                            else:
                                nc.vector.reduce_max(x1[:, k:k + 1], at[:], axis=AX.X)
                        if len(achunks) > 1:
                            nc.vector.reduce_max(x2[:, 0:1], x1[:, 0:len(achunks)],
                                                 axis=AX.X, negate=True)
                        # transpose -max into the g_a bias row; values land on
                        # rows 32b (aligned partition bases) after the 32x32
                        # block transpose since they sit in column 0.
                        xt = cm_pool.tile([128, 32], dt.float16, tag="xt")
                        nc.vector.transpose(xt[:], x2[:])
                        for b in range(4):
                            nc.vector.tensor_copy(
                                ga[C:C + 1, sub * 128 + 32 * b: sub * 128 + 32 * b + 32],
                                xt[32 * b: 32 * b + 1, 0:32])
                    # --- B + exp + C ---
                    G = gg.tile([C + 1, Q], dt.float32, tag="g")
                    mtiles = list(range(NT))
                    groups = [mtiles[i:i + 3] for i in range(0, NT, 3)]
                    for gi, grp in enumerate(groups):
                        bt = pp.tile([128, 512 * len(grp)], dt.float32, tag="b")
                        et = et_pool.tile([128, 512 * len(grp)], dt.float16, tag="et")
                        for jj, j in enumerate(grp):
                            nc.tensor.matmul(bt[:, jj * 512: jj * 512 + Q],
                                             fa[:, j * 128:(j + 1) * 128], ga[:],
                                             start=True, stop=True)
                        if Q == 512:
                            nc.scalar.activation(et[:], bt[:], AF.Exp)
                        else:
                            for jj in range(len(grp)):
                                nc.scalar.activation(et[:, jj * 512:jj * 512 + Q],
                                                     bt[:, jj * 512:jj * 512 + Q], AF.Exp)
                        for jj, j in enumerate(grp):
                            nc.tensor.matmul(G[:], fT[:, j * 65:(j + 1) * 65],
                                             et[:, jj * 512:jj * 512 + Q],
                                             start=(gi == 0 and jj == 0),
                                             stop=(j == NT - 1))
                    # --- D: normalize ---
                    Gs = dd_pool.tile([C, 512], dt.float32, tag="gs")
                    nc.vector.tensor_copy(Gs[:, 0:Q], G[0:C, :])
                    linv = dd_pool.tile([1, 512], dt.float16, tag="linv")
                    with nc.allow_low_precision(reason="softmax denom in fp16"):
                        nc.vector.reciprocal(linv[:, 0:Q], G[C:C + 1, :])
                    rep = rr.tile([C, Q], dt.float32, tag="r")
                    nc.tensor.matmul(rep[:], ones_rep[:], linv[0:1, 0:Q], start=True, stop=True)
                    write_out(isb, q0, Q, Gs, rep)

            # ---------------- scale 4 ----------------
            fT4 = build_fT(f4a, 2, "fT4")

            def w4(isb, q0, Q, Gs, rep):
                view = _v(att4p[:], 18 + 1, [[18, 16], [1, 16]])
                nc.vector.tensor_tensor(view, Gs[:, 0:Q], rep[:], op=ALU.mult)

            attn(f4a, fT4, 256, w4)

            # ---------------- scale 2 ----------------
            fT2 = build_fT(f2a, 8, "fT2")

            def w2(isb, q0, Q, Gs, rep):
                r0 = isb * 16
                view = _v(att2p[:], (1 + r0) * 34 + 1, [[34, 16], [1, 32]])
                nc.vector.tensor_tensor(view, Gs[:, 0:Q], rep[:], op=ALU.mult)

            attn(f2a, fT2, 1024, w2)

            # ---------------- upsample x4 into out_acc ----------------
            p4 = att4p[:]
            # edge replication (cols then rows so corners fill correctly)
            nc.vector.tensor_copy(_v(p4, 18, [[18, 16]]), _v(p4, 19, [[18, 16]]))
            nc.vector.tensor_copy(_v(p4, 18 + 17, [[18, 16]]), _v(p4, 18 + 16, [[18, 16]]))
            nc.vector.tensor_copy(_v(p4, 0, [[1, 18]]), _v(p4, 18, [[1, 18]]))
            nc.vector.tensor_copy(_v(p4, 17 * 18, [[1, 18]]), _v(p4, 16 * 18, [[1, 18]]))
            # W-stage: t4u rows 1..16 (padded layout [C, 18, 64])
            t4u = sb.tile([C, 18 * 64], dt.float32)
            pre58 = sb.tile([C, 256], dt.float32)   # 0.625 * center
            pre78 = sb.tile([C, 256], dt.float32)   # 0.875 * center
            ctr = _v(p4, 18 + 1, [[18, 16], [1, 16]])
            nc.vector.tensor_scalar_mul(pre58[:], ctr, 0.625)
            nc.vector.tensor_scalar_mul(pre78[:], ctr, 0.875)
            lft = _v(p4, 18 + 0, [[18, 16], [1, 16]])
            rgt = _v(p4, 18 + 2, [[18, 16], [1, 16]])
            for p, (nb, a, pre) in enumerate([(lft, 0.375, pre58), (lft, 0.125, pre78),
                                              (rgt, 0.125, pre78), (rgt, 0.375, pre58)]):
                outv = _v(t4u[:], 64 + p, [[64, 16], [4, 16]])
                nc.vector.scalar_tensor_tensor(outv, nb, a, pre[:], op0=ALU.mult, op1=ALU.add)
            nc.vector.tensor_copy(_v(t4u[:], 0, [[1, 64]]), _v(t4u[:], 64, [[1, 64]]))
            nc.vector.tensor_copy(_v(t4u[:], 17 * 64, [[1, 64]]), _v(t4u[:], 16 * 64, [[1, 64]]))
            # H-stage accumulate into out_acc (rows I = 4r+p)
            for p, (o1, a1, o2, a2) in enumerate([(0, 0.375, 64, 0.625), (0, 0.125, 64, 0.875),
                                                  (64, 0.875, 128, 0.125), (64, 0.625, 128, 0.375)]):
                outv = _v(out_acc[:], p * 64, [[256, 16], [1, 64]])
                for off, coef in ((o1, a1), (o2, a2)):
                    inv = _v(t4u[:], off, [[64, 16], [1, 64]])
                    nc.vector.scalar_tensor_tensor(outv, inv, coef, outv, op0=ALU.mult, op1=ALU.add)

            # ---------------- upsample x2 into out_acc ----------------
            p2 = att2p[:]
            nc.vector.tensor_copy(_v(p2, 34, [[34, 32]]), _v(p2, 35, [[34, 32]]))
            nc.vector.tensor_copy(_v(p2, 34 + 33, [[34, 32]]), _v(p2, 34 + 32, [[34, 32]]))
            nc.vector.tensor_copy(_v(p2, 0, [[1, 34]]), _v(p2, 34, [[1, 34]]))
            nc.vector.tensor_copy(_v(p2, 33 * 34, [[1, 34]]), _v(p2, 32 * 34, [[1, 34]]))
            t2u = sb.tile([C, 34 * 64], dt.float32)
            pre34 = sb.tile([C, 1024], dt.float32)  # 0.75 * center
            ctr2 = _v(p2, 34 + 1, [[34, 32], [1, 32]])
            nc.vector.tensor_scalar_mul(pre34[:], ctr2, 0.75)
            lft2 = _v(p2, 34 + 0, [[34, 32], [1, 32]])
            rgt2 = _v(p2, 34 + 2, [[34, 32], [1, 32]])
            for p, nb in enumerate([lft2, rgt2]):
                outv = _v(t2u[:], 64 + p, [[64, 32], [2, 32]])
                nc.vector.scalar_tensor_tensor(outv, nb, 0.25, pre34[:], op0=ALU.mult, op1=ALU.add)
            nc.vector.tensor_copy(_v(t2u[:], 0, [[1, 64]]), _v(t2u[:], 64, [[1, 64]]))
            nc.vector.tensor_copy(_v(t2u[:], 33 * 64, [[1, 64]]), _v(t2u[:], 32 * 64, [[1, 64]]))
            for p, (o1, a1, o2, a2) in enumerate([(0, 0.25, 64, 0.75), (64, 0.75, 128, 0.25)]):
                outv = _v(out_acc[:], p * 64, [[128, 32], [1, 64]])
                for off, coef in ((o1, a1), (o2, a2)):
                    inv = _v(t2u[:], off, [[64, 32], [1, 64]])
                    nc.vector.scalar_tensor_tensor(outv, inv, coef, outv, op0=ALU.mult, op1=ALU.add)

            # ---------------- scale 1 ----------------
            fT1 = build_fT(f1a, 32, "fT1")

            def w1(isb, q0, Q, Gs, rep):
                tmp = dd_pool.tile([C, 512], dt.float32, tag="tmp")
                nc.vector.tensor_tensor(tmp[:, 0:Q], Gs[:, 0:Q], rep[:], op=ALU.mult)
                nc.vector.tensor_tensor(out_acc[:, q0:q0 + Q], out_acc[:, q0:q0 + Q],
                                        tmp[:, 0:Q], op=ALU.add)

            attn(f1a, fT1, N1, w1)

            nc.sync.dma_start(out_d.ap(), out_acc[:])

    nc.compile()
    return nc


def _prep_inputs(x, W_std):
    lap = np.array([[0., 1., 0.], [1., -4., 1.], [0., 1., 0.]], dtype=np.float32)
    Wl = W_std.astype(np.float32) + lap[None, None] * np.eye(C, dtype=np.float32)[:, :, None, None]
    wt = np.ascontiguousarray(Wl.transpose(1, 2, 3, 0).reshape(C, 9 * C)).astype(np.float16)
    B = x.shape[0]
    xps = np.zeros((B, C, PAD, PAD), dtype=np.float16)
    xps[:, :, 1:H + 1, 1:W + 1] = x.astype(np.float16)
    return xps.reshape(B, C, PAD * PAD), wt


def _run(x, W_std, trace=False):
    x = np.asarray(x)
    W_std = np.asarray(W_std)
    xps, wt = _prep_inputs(x, W_std)
    if "nc" not in _cache:
        _cache["nc"] = _build_nc()
    nc = _cache["nc"]
    in_maps = [{"xp": np.ascontiguousarray(xps[i]), "wt": wt} for i in range(x.shape[0])]
    res = run_bass_kernel_spmd(nc, in_maps, core_ids=list(range(NCORES)), trace=trace)
    out = np.stack([res.results[i]["out"].reshape(C, H, W) for i in range(x.shape[0])])
    return out.astype(np.float32), res


def kernel(x, W_std):
    out, _ = _run(x, W_std, trace=False)
    return out


# revision 4
# speedup vs baseline: 1.2274x; 1.2274x over previous
"""Trainium2 Bass kernel for nn_BoundaryExtractionModule.

Data-parallel over batch: 8 samples -> 8 NeuronCores, one sample per core.

Per-core pipeline (channel-major layout [C, N] with C=64 on partitions):
  conv3x3(W_std)+depthwise-Laplacian   : 9 shift-matmuls per 512-col chunk
                                         (Laplacian folded into the taps on host)
  3-scale pooled non-local attention   : for each scale s in (4, 2, 1):
      A: row-max of logits  S = f^T f   (fp16 matmuls, DVE reduce_max)
      B: recompute S^T with the shift folded in via an augmented
         contraction row (K=65):  S'[m,q] = sum_k f_a[k,m] g_a[k,q]
         where f_a = [f; 1], g_a = [f; -rowmax]
      exp on ACT (PSUM -> fp16 SBUF)   : E^T tiles
      C: PV matmul with ones-column    : G = [f; 1] @ E^T  ->  G[64] = softmax denom
      D: out = G[0:64] * (1/G[64])     : gpsimd partition_broadcast + multiply
  bilinear x2/x4 upsample (half-pixel) : strided gpsimd ops on edge-padded buffers
  residual add + DMA out.

Engine split: PE matmuls; ACT exp + conv PSUM eviction; DVE row-max reduces,
PSUM->SBUF copies, reciprocal; GPSIMD pooling, upsampling, g_a assembly,
normalize-multiplies, residual.
"""

import numpy as np

import concourse.bass as bass
import concourse.mybir as mybir
import concourse.tile as tile
from concourse import bacc
from concourse.bass_utils import run_bass_kernel_spmd
from concourse.masks import make_identity

dt = mybir.dt
AF = mybir.ActivationFunctionType
ALU = mybir.AluOpType
AX = mybir.AxisListType

C = 64
H = W = 64
N1 = H * W          # 4096
PAD = 66            # padded row length for conv
NCORES = 8

_cache = {}


def _v(ap, off, dims):
    """View of `ap` at free-offset `off` with free dims `dims` (keeps partition dim)."""
    return bass.AP(ap.tensor, ap.offset + off, [list(ap.ap[0])] + [list(d) for d in dims])


def _chunks(total, size):
    out = []
    off = 0
    while off < total:
        out.append((off, min(size, total - off)))
        off += size
    return out


def _build_nc():
    nc = bacc.Bacc(None, target_bir_lowering=False)
    xp_d = nc.dram_tensor("xp", [C, PAD * PAD], dt.float16, kind="ExternalInput")
    wt_d = nc.dram_tensor("wt", [C, 9 * C], dt.float16, kind="ExternalInput")
    out_d = nc.dram_tensor("out", [C, N1], dt.float32, kind="ExternalOutput")

    with tile.TileContext(nc) as tc:
        with (
            tc.tile_pool(name="sb", bufs=1) as sb,
            tc.tile_pool(name="ga", bufs=2) as ga_pool,
            tc.tile_pool(name="et", bufs=3) as et_pool,
            tc.tile_pool(name="dd", bufs=2) as dd_pool,
            tc.tile_pool(name="cm", bufs=2) as cm_pool,
            tc.tile_pool(name="aa", bufs=3, space="PSUM") as aa,
            tc.tile_pool(name="pp", bufs=2, space="PSUM") as pp,
            tc.tile_pool(name="gg", bufs=1, space="PSUM") as gg,
        ):
            # ---------------- inputs / constants ----------------
            xp16 = sb.tile([C, PAD * PAD], dt.float16)
            nc.sync.dma_start(xp16[:], xp_d.ap())
            wt16 = sb.tile([C, 9 * C], dt.float16)
            nc.sync.dma_start(wt16[:], wt_d.ap())

            ident = sb.tile([128, 128], dt.float16)
            make_identity(nc, ident[:])

            out_acc = sb.tile([C, N1], dt.float32)
            # residual init: out_acc = x  (from the padded fp16 input)
            nc.gpsimd.tensor_copy(out_acc[:], _v(xp16[:], PAD + 1, [[PAD, H], [1, W]]))

            # ---------------- conv (feat -> f1a) ----------------
            f1a = sb.tile([C + 1, N1], dt.float16)
            for r in range(8):
                cp = pp.tile([C, 512], dt.float32, tag="b")
                for tap in range(9):
                    dy, dx = divmod(tap, 3)
                    rhs = _v(xp16[:], (8 * r + dy) * PAD + dx, [[PAD, 8], [1, W]])
                    nc.tensor.matmul(cp[:], wt16[:, tap * C:(tap + 1) * C], rhs,
                                     start=(tap == 0), stop=(tap == 8))
                nc.scalar.copy(f1a[0:C, r * 512:(r + 1) * 512], cp[:])
            nc.vector.memset(f1a[C:C + 1, :], 1.0)

            # ---------------- pools (f2a, f4a) on gpsimd ----------------
            f1 = f1a[0:C, :]
            t2w = sb.tile([C, 2048], dt.float32)
            nc.gpsimd.tensor_tensor(t2w[:], _v(f1, 0, [[2, 2048]]), _v(f1, 1, [[2, 2048]]), op=ALU.add)
            f2raw = sb.tile([C, 1024], dt.float32)
            nc.gpsimd.tensor_tensor(f2raw[:], _v(t2w[:], 0, [[64, 32], [1, 32]]),
                                    _v(t2w[:], 32, [[64, 32], [1, 32]]), op=ALU.add)
            f2a = sb.tile([C + 1, 1024], dt.float16)
            nc.gpsimd.tensor_scalar_mul(f2a[0:C, :], f2raw[:], 0.25)
            nc.gpsimd.memset(f2a[C:C + 1, :], 1.0)

            t4w = sb.tile([C, 512], dt.float32)
            nc.gpsimd.tensor_tensor(t4w[:], _v(f2raw[:], 0, [[2, 512]]), _v(f2raw[:], 1, [[2, 512]]), op=ALU.add)
            f4raw = sb.tile([C, 256], dt.float32)
            nc.gpsimd.tensor_tensor(f4raw[:], _v(t4w[:], 0, [[32, 16], [1, 16]]),
                                    _v(t4w[:], 16, [[32, 16], [1, 16]]), op=ALU.add)
            f4a = sb.tile([C + 1, 256], dt.float16)
            nc.gpsimd.tensor_scalar_mul(f4a[0:C, :], f4raw[:], 1.0 / 16.0)
            nc.gpsimd.memset(f4a[C:C + 1, :], 1.0)

            att2p = sb.tile([C, 34 * 34], dt.float32)   # scale-2 attn out, 1-px padded
            att4p = sb.tile([C, 18 * 18], dt.float32)   # scale-4 attn out, 1-px padded

            # ---------------- generic attention ----------------
            def build_fT(fa, NT, name):
                fT = sb.tile([128, NT * 65], dt.float16, tag=name)
                nc.vector.memset(_v(fT[:], C, [[65, NT]]), 1.0)
                for j in range(NT):
                    pt = pp.tile([128, C], dt.float16, tag="b")
                    nc.tensor.transpose(pt[:], fa[0:C, j * 128:(j + 1) * 128], ident[0:C, 0:C])
                    nc.vector.tensor_copy(fT[:, j * 65:j * 65 + C], pt[:])
                return fT

            def attn(fa, fT, N, write_out):
                NT = N // 128
                for isb, (q0, Q) in enumerate(_chunks(N, 512)):
                    nsub = Q // 128
                    ga = ga_pool.tile([C + 1, Q], dt.float16, tag="ga")
                    nc.gpsimd.tensor_copy(ga[0:C, :], fa[0:C, q0:q0 + Q])
                    # --- A: row maxes ---
                    for sub in range(nsub):
                        lhsA = fa[0:C, q0 + sub * 128: q0 + (sub + 1) * 128]
                        achunks = _chunks(N, 512)
                        x2 = cm_pool.tile([128, 32], dt.float16, tag="x2")
                        if len(achunks) > 1:
                            x1 = cm_pool.tile([128, 8], dt.float32, tag="x1")
                        for k, (off, ln) in enumerate(achunks):
                            at = aa.tile([128, ln], dt.float32, tag="a")
                            nc.tensor.matmul(at[:], lhsA, fa[0:C, off:off + ln],
                                             start=True, stop=True)
                            if len(achunks) == 1:
                                nc.vector.reduce_max(x2[:, 0:1], at[:], axis=AX.X, negate=True)
                            else:
                                nc.vector.reduce_max(x1[:, k:k + 1], at[:], axis=AX.X)
                        if len(achunks) > 1:
                            nc.vector.reduce_max(x2[:, 0:1], x1[:, 0:len(achunks)],
                                                 axis=AX.X, negate=True)
                        # transpose -max into the g_a bias row; values land on
                        # rows 32b (aligned partition bases) after the 32x32
                        # block transpose since they sit in column 0.
                        xt = cm_pool.tile([128, 32], dt.float16, tag="xt")
                        nc.vector.transpose(xt[:], x2[:])
                        for b in range(4):
                            nc.gpsimd.tensor_copy(
                                ga[C:C + 1, sub * 128 + 32 * b: sub * 128 + 32 * b + 32],
                                xt[32 * b: 32 * b + 1, 0:32])
                    # --- B + exp + C ---
                    G = gg.tile([C + 1, Q], dt.float32, tag="g")
                    mtiles = list(range(NT))
                    groups = [mtiles[i:i + 2] for i in range(0, NT, 2)]
                    for gi, grp in enumerate(groups):
                        bt = pp.tile([128, 512 * len(grp)], dt.float32, tag="b")
                        et = et_pool.tile([128, 512 * len(grp)], dt.float16, tag="et")
                        for jj, j in enumerate(grp):
                            nc.tensor.matmul(bt[:, jj * 512: jj * 512 + Q],
                                             fa[:, j * 128:(j + 1) * 128], ga[:],
                                             start=True, stop=True)
                        if Q == 512:
                            nc.scalar.activation(et[:], bt[:], AF.Exp)
                        else:
                            for jj in range(len(grp)):
                                nc.scalar.activation(et[:, jj * 512:jj * 512 + Q],
                                                     bt[:, jj * 512:jj * 512 + Q], AF.Exp)
                        for jj, j in enumerate(grp):
                            nc.tensor.matmul(G[:], fT[:, j * 65:(j + 1) * 65],
                                             et[:, jj * 512:jj * 512 + Q],
                                             start=(gi == 0 and jj == 0),
                                             stop=(j == NT - 1))
                    # --- D: normalize ---
                    Gs = dd_pool.tile([C + 1, 512], dt.float32, tag="gs")
                    nc.vector.tensor_copy(Gs[:, 0:Q], G[:])
                    linv = dd_pool.tile([1, 512], dt.float16, tag="linv")
                    with nc.allow_low_precision(reason="softmax denom in fp16"):
                        nc.vector.reciprocal(linv[:, 0:Q], Gs[C:C + 1, 0:Q])
                    lrep = dd_pool.tile([C, 512], dt.float16, tag="lrep")
                    nc.gpsimd.partition_broadcast(lrep[:, 0:Q], linv[0:1, 0:Q])
                    write_out(isb, q0, Q, Gs, lrep)

            # ---------------- scale 4 ----------------
            fT4 = build_fT(f4a, 2, "fT4")

            def w4(isb, q0, Q, Gs, lrep):
                view = _v(att4p[:], 18 + 1, [[18, 16], [1, 16]])
                nc.gpsimd.tensor_tensor(view, Gs[0:C, 0:Q], lrep[:, 0:Q], op=ALU.mult)

            attn(f4a, fT4, 256, w4)

            # ---------------- scale 2 ----------------
            fT2 = build_fT(f2a, 8, "fT2")

            def w2(isb, q0, Q, Gs, lrep):
                r0 = isb * 16
                view = _v(att2p[:], (1 + r0) * 34 + 1, [[34, 16], [1, 32]])
                nc.gpsimd.tensor_tensor(view, Gs[0:C, 0:Q], lrep[:, 0:Q], op=ALU.mult)

            attn(f2a, fT2, 1024, w2)

            # ---------------- upsample x4 into out_acc (gpsimd) ----------------
            ups = sb.tile([C, 2048], dt.float32)    # shared gpsimd scratch
            p4 = att4p[:]
            # edge replication (cols then rows so corners fill correctly)
            nc.gpsimd.tensor_copy(_v(p4, 18, [[18, 16]]), _v(p4, 19, [[18, 16]]))
            nc.gpsimd.tensor_copy(_v(p4, 18 + 17, [[18, 16]]), _v(p4, 18 + 16, [[18, 16]]))
            nc.gpsimd.tensor_copy(_v(p4, 0, [[1, 18]]), _v(p4, 18, [[1, 18]]))
            nc.gpsimd.tensor_copy(_v(p4, 17 * 18, [[1, 18]]), _v(p4, 16 * 18, [[1, 18]]))
            # W-stage: t4u rows 1..16 (padded layout [C, 18, 64])
            t4u = sb.tile([C, 18 * 64], dt.float32)
            pre58 = sb.tile([C, 256], dt.float32)   # 0.625 * center
            pre78 = sb.tile([C, 256], dt.float32)   # 0.875 * center
            ctr = _v(p4, 18 + 1, [[18, 16], [1, 16]])
            nc.gpsimd.tensor_scalar_mul(pre58[:], ctr, 0.625)
            nc.gpsimd.tensor_scalar_mul(pre78[:], ctr, 0.875)
            lft = _v(p4, 18 + 0, [[18, 16], [1, 16]])
            rgt = _v(p4, 18 + 2, [[18, 16], [1, 16]])
            for p, (nb, a, pre) in enumerate([(lft, 0.375, pre58), (lft, 0.125, pre78),
                                              (rgt, 0.125, pre78), (rgt, 0.375, pre58)]):
                outv = _v(t4u[:], 64 + p, [[64, 16], [4, 16]])
                sc = _v(ups[:], 0, [[1, 256]])
                nc.gpsimd.tensor_scalar_mul(sc, nb, a)
                nc.gpsimd.tensor_tensor(outv, sc, pre[:], op=ALU.add)
            nc.gpsimd.tensor_copy(_v(t4u[:], 0, [[1, 64]]), _v(t4u[:], 64, [[1, 64]]))
            nc.gpsimd.tensor_copy(_v(t4u[:], 17 * 64, [[1, 64]]), _v(t4u[:], 16 * 64, [[1, 64]]))
            # H-stage accumulate into out_acc (rows I = 4r+p)
            for p, (o1, a1, o2, a2) in enumerate([(0, 0.375, 64, 0.625), (0, 0.125, 64, 0.875),
                                                  (64, 0.875, 128, 0.125), (64, 0.625, 128, 0.375)]):
                outv = _v(out_acc[:], p * 64, [[256, 16], [1, 64]])
                for off, coef in ((o1, a1), (o2, a2)):
                    inv = _v(t4u[:], off, [[64, 16], [1, 64]])
                    sc = _v(ups[:], 0, [[1, 1024]])
                    nc.gpsimd.tensor_scalar_mul(sc, inv, coef)
                    nc.gpsimd.tensor_tensor(outv, outv, sc, op=ALU.add)

            # ---------------- upsample x2 into out_acc (gpsimd) ----------------
            p2 = att2p[:]
            nc.gpsimd.tensor_copy(_v(p2, 34, [[34, 32]]), _v(p2, 35, [[34, 32]]))
            nc.gpsimd.tensor_copy(_v(p2, 34 + 33, [[34, 32]]), _v(p2, 34 + 32, [[34, 32]]))
            nc.gpsimd.tensor_copy(_v(p2, 0, [[1, 34]]), _v(p2, 34, [[1, 34]]))
            nc.gpsimd.tensor_copy(_v(p2, 33 * 34, [[1, 34]]), _v(p2, 32 * 34, [[1, 34]]))
            t2u = sb.tile([C, 34 * 64], dt.float32)
            pre34 = sb.tile([C, 1024], dt.float32)  # 0.75 * center
            ctr2 = _v(p2, 34 + 1, [[34, 32], [1, 32]])
            nc.gpsimd.tensor_scalar_mul(pre34[:], ctr2, 0.75)
            lft2 = _v(p2, 34 + 0, [[34, 32], [1, 32]])
            rgt2 = _v(p2, 34 + 2, [[34, 32], [1, 32]])
            for p, nb in enumerate([lft2, rgt2]):
                outv = _v(t2u[:], 64 + p, [[64, 32], [2, 32]])
                sc = _v(ups[:], 0, [[1, 1024]])
                nc.gpsimd.tensor_scalar_mul(sc, nb, 0.25)
                nc.gpsimd.tensor_tensor(outv, sc, pre34[:], op=ALU.add)
            nc.gpsimd.tensor_copy(_v(t2u[:], 0, [[1, 64]]), _v(t2u[:], 64, [[1, 64]]))
            nc.gpsimd.tensor_copy(_v(t2u[:], 33 * 64, [[1, 64]]), _v(t2u[:], 32 * 64, [[1, 64]]))
            for p, (o1, a1, o2, a2) in enumerate([(0, 0.25, 64, 0.75), (64, 0.75, 128, 0.25)]):
                outv = _v(out_acc[:], p * 64, [[128, 32], [1, 64]])
                for off, coef in ((o1, a1), (o2, a2)):
                    inv = _v(t2u[:], off, [[64, 32], [1, 64]])
                    sc = _v(ups[:], 0, [[1, 2048]])
                    nc.gpsimd.tensor_scalar_mul(sc, inv, coef)
                    nc.gpsimd.tensor_tensor(outv, outv, sc, op=ALU.add)

            # ---------------- scale 1 ----------------
            fT1 = build_fT(f1a, 32, "fT1")

            def w1(isb, q0, Q, Gs, lrep):
                tmp = dd_pool.tile([C, 512], dt.float32, tag="tmp")
                nc.gpsimd.tensor_tensor(tmp[:, 0:Q], Gs[0:C, 0:Q], lrep[:, 0:Q], op=ALU.mult)
                nc.gpsimd.tensor_tensor(out_acc[:, q0:q0 + Q], out_acc[:, q0:q0 + Q],
                                        tmp[:, 0:Q], op=ALU.add)

            attn(f1a, fT1, N1, w1)

            nc.sync.dma_start(out_d.ap(), out_acc[:])

    nc.compile()
    return nc


def _prep_inputs(x, W_std):
    lap = np.array([[0., 1., 0.], [1., -4., 1.], [0., 1., 0.]], dtype=np.float32)
    Wl = W_std.astype(np.float32) + lap[None, None] * np.eye(C, dtype=np.float32)[:, :, None, None]
    wt = np.ascontiguousarray(Wl.transpose(1, 2, 3, 0).reshape(C, 9 * C)).astype(np.float16)
    B = x.shape[0]
    xps = np.zeros((B, C, PAD, PAD), dtype=np.float16)
    xps[:, :, 1:H + 1, 1:W + 1] = x.astype(np.float16)
    return xps.reshape(B, C, PAD * PAD), wt


def _run(x, W_std, trace=False):
    x = np.asarray(x)
    W_std = np.asarray(W_std)
    xps, wt = _prep_inputs(x, W_std)
    if "nc" not in _cache:
        _cache["nc"] = _build_nc()
    nc = _cache["nc"]
    in_maps = [{"xp": np.ascontiguousarray(xps[i]), "wt": wt} for i in range(x.shape[0])]
    res = run_bass_kernel_spmd(nc, in_maps, core_ids=list(range(NCORES)), trace=trace)
    out = np.stack([res.results[i]["out"].reshape(C, H, W) for i in range(x.shape[0])])
    return out.astype(np.float32), res


def kernel(x, W_std):
    out, _ = _run(x, W_std, trace=False)
    return out


# revision 69
# speedup vs baseline: 1.6453x; 1.3405x over previous
"""Trainium2 Bass kernel for nn_BoundaryExtractionModule.

Data-parallel over batch: 8 samples -> 8 NeuronCores, one sample per core.

Per-core pipeline (channel-major layout [C, N] with C=64 on partitions):
  conv3x3(W_std)+depthwise-Laplacian   : 9 shift-matmuls per 512-col chunk
                                         (Laplacian folded into the taps on host)
  3-scale pooled non-local attention   : for each scale s in (4, 2, 1):
      A: row-max of logits  S = f^T f   (fp16 matmuls, DVE reduce_max)
      B: recompute S^T with the shift folded in via an augmented
         contraction row (K=65):  S'[m,q] = sum_k f_a[k,m] g_a[k,q]
         where f_a = [f; 1], g_a = [f; -rowmax]
      exp on ACT (PSUM -> fp16 SBUF)   : E^T tiles
      C: PV matmul with ones-column    : G = [f; 1] @ E^T  ->  G[64] = softmax denom
      D: out = G[0:64] * (1/G[64])     : gpsimd partition_broadcast + multiply
  bilinear x2/x4 upsample (half-pixel) : strided ops on edge-padded buffers
  residual add + DMA out.

The emission order interleaves the small scales and upsampling into scale-1's
superblock stream so every engine stays busy (Tile schedules greedily in
program order).
"""

import numpy as np

import concourse.bass as bass
import concourse.mybir as mybir
import concourse.tile as tile
from concourse import bacc
from concourse.bass_utils import run_bass_kernel_spmd
from concourse.masks import make_identity

dt = mybir.dt
AF = mybir.ActivationFunctionType
ALU = mybir.AluOpType
AX = mybir.AxisListType

C = 64
H = W = 64
N1 = H * W          # 4096
PAD = 66            # padded row length for conv
NCORES = 8

_cache = {}


def _v(ap, off, dims):
    """View of `ap` at free-offset `off` with free dims `dims` (keeps partition dim)."""
    return bass.AP(ap.tensor, ap.offset + off, [list(ap.ap[0])] + [list(d) for d in dims])


def _chunks(total, size):
    out = []
    off = 0
    while off < total:
        out.append((off, min(size, total - off)))
        off += size
    return out


def _build_nc():
    nc = bacc.Bacc(None, target_bir_lowering=False)
    xp_d = nc.dram_tensor("xp", [C, PAD * PAD], dt.float16, kind="ExternalInput")
    wt_d = nc.dram_tensor("wt", [C, 9 * C], dt.float16, kind="ExternalInput")
    out_d = nc.dram_tensor("out", [C, N1], dt.float32, kind="ExternalOutput")

    with tile.TileContext(nc) as tc:
        with (
            tc.tile_pool(name="sb", bufs=1) as sb,
            tc.tile_pool(name="ga", bufs=4) as ga_pool,
            tc.tile_pool(name="et", bufs=6) as et_pool,
            tc.tile_pool(name="dd", bufs=4) as dd_pool,
            tc.tile_pool(name="cm", bufs=18) as cm_pool,
            tc.tile_pool(name="aa", bufs=3, space="PSUM") as aa,
            tc.tile_pool(name="pp", bufs=2, space="PSUM") as pp,
            tc.tile_pool(name="gg", bufs=1, space="PSUM") as gg,
        ):
            # ---------------- inputs / constants ----------------
            xp16 = sb.tile([C, PAD * PAD], dt.float16)
            # split the input DMA so conv chunk 0 (rows 0..9) can start early
            nc.sync.dma_start(xp16[:, 0:10 * PAD], xp_d.ap()[:, 0:10 * PAD])
            nc.sync.dma_start(xp16[:, 10 * PAD:], xp_d.ap()[:, 10 * PAD:])
            wt16 = sb.tile([C, 9 * C], dt.float16)
            nc.sync.dma_start(wt16[:], wt_d.ap())

            ident = sb.tile([128, 128], dt.float16)
            make_identity(nc, ident[:])

            out_acc = sb.tile([C, N1], dt.float32)
            # residual init: out_acc = x  (from the padded fp16 input)
            nc.gpsimd.tensor_copy(out_acc[:], _v(xp16[:], PAD + 1, [[PAD, H], [1, W]]))

            f1a = sb.tile([C + 1, N1], dt.float16)
            fT1 = sb.tile([128, 32 * 65], dt.float16)
            nc.vector.memset(_v(fT1[:], C, [[65, 32]]), 1.0)
            nc.vector.memset(f1a[C:C + 1, :], 1.0)

            # ---------------- generic attention (per-superblock emitter) ----------------
            def build_fT(fa, NT, name):
                fT = sb.tile([128, NT * 65], dt.float16, tag=name)
                nc.vector.memset(_v(fT[:], C, [[65, NT]]), 1.0)
                for j in range(NT):
                    pt = pp.tile([128, C], dt.float16, tag="b")
                    nc.tensor.transpose(pt[:], fa[0:C, j * 128:(j + 1) * 128], ident[0:C, 0:C])
                    nc.scalar.copy(fT[:, j * 65:j * 65 + C], pt[:])
                return fT

            def _achunks(N):
                return _chunks(N, 512)

            def attn_A_start(fa, N, isb):
                q0 = isb * 512
                Q = min(512, N - q0)
                nsub = Q // 128
                achunks = _achunks(N)
                multi = len(achunks) > 1
                return dict(
                    fa=fa, N=N, isb=isb, q0=q0, Q=Q, nsub=nsub, achunks=achunks,
                    x1=[cm_pool.tile([128, 8], dt.float32, tag="x1", name=f"x1_{isb}_{s}")
                        for s in range(nsub)] if multi else None,
                    x2=[cm_pool.tile([128, 1], dt.float16, tag="x2", name=f"x2_{isb}_{s}")
                        for s in range(nsub)],
                )

            def attn_A_chunk(st, k):
                fa, q0 = st["fa"], st["q0"]
                off, ln = st["achunks"][k]
                for sub in range(st["nsub"]):
                    lhsA = fa[0:C, q0 + sub * 128: q0 + (sub + 1) * 128]
                    at = aa.tile([128, ln], dt.float32, tag="a")
                    for h0, hl in _chunks(ln, 512):
                        nc.tensor.matmul(at[:, h0:h0 + hl], lhsA,
                                         fa[0:C, off + h0:off + h0 + hl],
                                         start=True, stop=True)
                    if st["x1"] is None:
                        nc.vector.reduce_max(st["x2"][sub][:], at[:], axis=AX.X, negate=True)
                    else:
                        nc.vector.reduce_max(st["x1"][sub][:, k:k + 1], at[:], axis=AX.X)

            def attn_finish(st, fT, write_out, filler=()):
                fa, N, isb = st["fa"], st["N"], st["isb"]
                q0, Q, nsub = st["q0"], st["Q"], st["nsub"]
                NT = N // 128
                nch = len(st["achunks"])
                ga = ga_pool.tile([C + 1, Q], dt.float16, tag="ga")
                nc.vector.tensor_copy(ga[0:C, :], fa[0:C, q0:q0 + Q])
                for sub in range(nsub):
                    x2 = st["x2"][sub]
                    if st["x1"] is not None:
                        nc.vector.reduce_max(x2[:], st["x1"][sub][:, 0:nch],
                                             axis=AX.X, negate=True)
                    # PE-transpose -max [128,1] -> [1,128] into the g_a bias row
                    pt = aa.tile([1, 128], dt.float16, tag="a")
                    nc.tensor.transpose(pt[:], x2[:], ident[:])
                    nc.vector.tensor_copy(ga[C:C + 1, sub * 128:(sub + 1) * 128], pt[:])
                # --- B + exp + C (filler thunks keep PE fed while exp runs) ---
                G = gg.tile([C + 1, Q], dt.float32, tag="g")
                mtiles = list(range(NT))
                groups = [mtiles[i:i + 2] for i in range(0, NT, 2)]
                filler = list(filler)
                fill_at = {int(i * len(groups) / len(filler)): i for i in range(len(filler))} if filler else {}
                for gi, grp in enumerate(groups):
                    if gi in fill_at:
                        filler[fill_at[gi]]()
                    bt = pp.tile([128, 512 * len(grp)], dt.float32, tag="b")
                    et = et_pool.tile([128, 512 * len(grp)], dt.float16, tag="et")
                    for jj, j in enumerate(grp):
                        nc.tensor.matmul(bt[:, jj * 512: jj * 512 + Q],
                                         fa[:, j * 128:(j + 1) * 128], ga[:],
                                         start=True, stop=True)
                    if Q == 512:
                        nc.scalar.activation(et[:], bt[:], AF.Exp)
                    else:
                        for jj in range(len(grp)):
                            nc.scalar.activation(et[:, jj * 512:jj * 512 + Q],
                                                 bt[:, jj * 512:jj * 512 + Q], AF.Exp)
                    for jj, j in enumerate(grp):
                        nc.tensor.matmul(G[:], fT[:, j * 65:(j + 1) * 65],
                                         et[:, jj * 512:jj * 512 + Q],
                                         start=(gi == 0 and jj == 0),
                                         stop=(j == NT - 1))
                # --- D: normalize ---
                Gs = dd_pool.tile([C + 1, 512], dt.float32, tag="gs")
                nc.vector.tensor_copy(Gs[:, 0:Q], G[:])
                linv = dd_pool.tile([1, 512], dt.float32, tag="linv")
                nc.vector.reciprocal(linv[:, 0:Q], Gs[C:C + 1, 0:Q])
                lrep = dd_pool.tile([C, 512], dt.float32, tag="lrep")
                nc.gpsimd.partition_broadcast(lrep[:, 0:Q], linv[0:1, 0:Q])
                write_out(isb, q0, Q, Gs, lrep)

            def w1(isb, q0, Q, Gs, lrep):
                eng = nc.gpsimd
                tmp = dd_pool.tile([C, 512], dt.float32, tag="tmp")
                eng.tensor_tensor(tmp[:, 0:Q], Gs[0:C, 0:Q], lrep[:, 0:Q], op=ALU.mult)
                eng.tensor_tensor(out_acc[:, q0:q0 + Q], out_acc[:, q0:q0 + Q],
                                  tmp[:, 0:Q], op=ALU.add)

            att2p = sb.tile([C, 34 * 34], dt.float32)   # scale-2 attn out, 1-px padded
            att4p = sb.tile([C, 18 * 18], dt.float32)   # scale-4 attn out, 1-px padded
            up_acc = sb.tile([C, N1], dt.float32)       # upsampled x2+x4 sum

            def w2(isb, q0, Q, Gs, lrep):
                r0 = isb * 16
                view = _v(att2p[:], (1 + r0) * 34 + 1, [[34, 16], [1, 32]])
                nc.gpsimd.tensor_tensor(view, Gs[0:C, 0:Q], lrep[:, 0:Q], op=ALU.mult)

            def w4(isb, q0, Q, Gs, lrep):
                view = _v(att4p[:], 18 + 1, [[18, 16], [1, 16]])
                nc.gpsimd.tensor_tensor(view, Gs[0:C, 0:Q], lrep[:, 0:Q], op=ALU.mult)

            # ---------------- pool emitters (gpsimd) ----------------
            f2raw = sb.tile([C, 1024], dt.float32)
            f2a = sb.tile([C + 1, 1024], dt.float16)
            f4a = sb.tile([C + 1, 256], dt.float16)

            def emit_pools2():
                f1 = f1a[0:C, :]
                t2w = sb.tile([C, 2048], dt.float32)
                nc.gpsimd.tensor_tensor(t2w[:], _v(f1, 0, [[2, 2048]]), _v(f1, 1, [[2, 2048]]), op=ALU.add)
                nc.gpsimd.tensor_tensor(f2raw[:], _v(t2w[:], 0, [[64, 32], [1, 32]]),
                                        _v(t2w[:], 32, [[64, 32], [1, 32]]), op=ALU.add)
                nc.gpsimd.tensor_scalar_mul(f2a[0:C, :], f2raw[:], 0.25)
                nc.gpsimd.memset(f2a[C:C + 1, :], 1.0)

            def emit_pools4():
                t4w = sb.tile([C, 512], dt.float32)
                nc.gpsimd.tensor_tensor(t4w[:], _v(f2raw[:], 0, [[2, 512]]), _v(f2raw[:], 1, [[2, 512]]), op=ALU.add)
                f4raw = sb.tile([C, 256], dt.float32)
                nc.gpsimd.tensor_tensor(f4raw[:], _v(t4w[:], 0, [[32, 16], [1, 16]]),
                                        _v(t4w[:], 16, [[32, 16], [1, 16]]), op=ALU.add)
                nc.gpsimd.tensor_scalar_mul(f4a[0:C, :], f4raw[:], 1.0 / 16.0)
                nc.gpsimd.memset(f4a[C:C + 1, :], 1.0)

            # ---------------- upsample emitters ----------------
            def emit_up4():
                p4 = att4p[:]
                ups = sb.tile([C, 256], dt.float32, tag="ups4")
                # edge replication (cols then rows so corners fill correctly)
                nc.gpsimd.tensor_copy(_v(p4, 18, [[18, 16]]), _v(p4, 19, [[18, 16]]))
                nc.gpsimd.tensor_copy(_v(p4, 18 + 17, [[18, 16]]), _v(p4, 18 + 16, [[18, 16]]))
                nc.gpsimd.tensor_copy(_v(p4, 0, [[1, 18]]), _v(p4, 18, [[1, 18]]))
                nc.gpsimd.tensor_copy(_v(p4, 17 * 18, [[1, 18]]), _v(p4, 16 * 18, [[1, 18]]))
                # W-stage: t4u rows 1..16 (padded layout [C, 18, 64]) on gpsimd
                t4u = sb.tile([C, 18 * 64], dt.float32)
                pre58 = sb.tile([C, 256], dt.float32)   # 0.625 * center
                pre78 = sb.tile([C, 256], dt.float32)   # 0.875 * center
                ctr = _v(p4, 18 + 1, [[18, 16], [1, 16]])
                nc.gpsimd.tensor_scalar_mul(pre58[:], ctr, 0.625)
                nc.gpsimd.tensor_scalar_mul(pre78[:], ctr, 0.875)
                lft = _v(p4, 18 + 0, [[18, 16], [1, 16]])
                rgt = _v(p4, 18 + 2, [[18, 16], [1, 16]])
                for p, (nb, a, pre) in enumerate([(lft, 0.375, pre58), (lft, 0.125, pre78),
                                                  (rgt, 0.125, pre78), (rgt, 0.375, pre58)]):
                    outv = _v(t4u[:], 64 + p, [[64, 16], [4, 16]])
                    nc.gpsimd.tensor_scalar_mul(ups[:], nb, a)
                    nc.gpsimd.tensor_tensor(outv, ups[:], pre[:], op=ALU.add)
                nc.gpsimd.tensor_copy(_v(t4u[:], 0, [[1, 64]]), _v(t4u[:], 64, [[1, 64]]))
                nc.gpsimd.tensor_copy(_v(t4u[:], 17 * 64, [[1, 64]]), _v(t4u[:], 16 * 64, [[1, 64]]))
                # H-stage into up_acc (rows I = 4r+p): first op writes, second accumulates
                u4s = sb.tile([C, 1024], dt.float32)
                for p, (o1, a1, o2, a2) in enumerate([(0, 0.375, 64, 0.625), (0, 0.125, 64, 0.875),
                                                      (64, 0.875, 128, 0.125), (64, 0.625, 128, 0.375)]):
                    outv = _v(up_acc[:], p * 64, [[256, 16], [1, 64]])
                    nc.gpsimd.tensor_scalar_mul(outv, _v(t4u[:], o1, [[64, 16], [1, 64]]), a1)
                    nc.gpsimd.tensor_scalar_mul(u4s[:], _v(t4u[:], o2, [[64, 16], [1, 64]]), a2)
                    nc.gpsimd.tensor_tensor(outv, outv, u4s[:], op=ALU.add)

            def emit_up2():
                p2 = att2p[:]
                ups = sb.tile([C, 1024], dt.float32, tag="ups2")
                nc.gpsimd.tensor_copy(_v(p2, 34, [[34, 32]]), _v(p2, 35, [[34, 32]]))
                nc.gpsimd.tensor_copy(_v(p2, 34 + 33, [[34, 32]]), _v(p2, 34 + 32, [[34, 32]]))
                nc.gpsimd.tensor_copy(_v(p2, 0, [[1, 34]]), _v(p2, 34, [[1, 34]]))
                nc.gpsimd.tensor_copy(_v(p2, 33 * 34, [[1, 34]]), _v(p2, 32 * 34, [[1, 34]]))
                t2u = sb.tile([C, 34 * 64], dt.float32)
                pre34 = sb.tile([C, 1024], dt.float32)  # 0.75 * center
                ctr2 = _v(p2, 34 + 1, [[34, 32], [1, 32]])
                nc.gpsimd.tensor_scalar_mul(pre34[:], ctr2, 0.75)
                lft2 = _v(p2, 34 + 0, [[34, 32], [1, 32]])
                rgt2 = _v(p2, 34 + 2, [[34, 32], [1, 32]])
                for p, nb in enumerate([lft2, rgt2]):
                    outv = _v(t2u[:], 64 + p, [[64, 32], [2, 32]])
                    nc.gpsimd.tensor_scalar_mul(ups[:], nb, 0.25)
                    nc.gpsimd.tensor_tensor(outv, ups[:], pre34[:], op=ALU.add)
                nc.gpsimd.tensor_copy(_v(t2u[:], 0, [[1, 64]]), _v(t2u[:], 64, [[1, 64]]))
                nc.gpsimd.tensor_copy(_v(t2u[:], 33 * 64, [[1, 64]]), _v(t2u[:], 32 * 64, [[1, 64]]))
                u2s = sb.tile([C, 2048], dt.float32)
                for p, (o1, a1, o2, a2) in enumerate([(0, 0.25, 64, 0.75), (64, 0.75, 128, 0.25)]):
                    outv = _v(up_acc[:], p * 64, [[128, 32], [1, 64]])
                    for off, coef in ((o1, a1), (o2, a2)):
                        nc.gpsimd.tensor_scalar_mul(u2s[:], _v(t2u[:], off, [[64, 32], [1, 64]]), coef)
                        nc.gpsimd.tensor_tensor(outv, outv, u2s[:], op=ALU.add)

            def attn_sb(fa, fT, N, isb, write_out):
                st = attn_A_start(fa, N, isb)
                for k in range(len(st["achunks"])):
                    attn_A_chunk(st, k)
                attn_finish(st, fT, write_out)

            # ---------------- master schedule ----------------
            # conv chunks interleaved with fT1 build and sb0/sb1's A-pass
            # (A-chunk k only needs conv chunk k evicted).
            st0 = attn_A_start(f1a, N1, 0)
            st1 = attn_A_start(f1a, N1, 1)
            # A-chunk (st, k) becomes runnable once conv has evicted its columns
            asched = {0: [(st0, 0)], 1: [(st1, 0), (st0, 1)], 2: [(st1, 1), (st0, 2)],
                      3: [(st1, 2), (st0, 3)], 4: [(st1, 3), (st0, 4)],
                      5: [(st1, 4), (st0, 5)], 6: [(st1, 5), (st0, 6)],
                      7: [(st1, 6), (st0, 7)]}
            for r in range(8):
                cp = pp.tile([C, 512], dt.float32, tag="b")
                for tap in range(9):
                    dy, dx = divmod(tap, 3)
                    rhs = _v(xp16[:], (8 * r + dy) * PAD + dx, [[PAD, 8], [1, W]])
                    nc.tensor.matmul(cp[:], wt16[:, tap * C:(tap + 1) * C], rhs,
                                     start=(tap == 0), stop=(tap == 8))
                nc.scalar.copy(f1a[0:C, r * 512:(r + 1) * 512], cp[:])
                for st, k in asched.get(r, []):
                    attn_A_chunk(st, k)
                for j in range(4 * r, 4 * r + 4):
                    pt = pp.tile([128, C], dt.float16, tag="b")
                    nc.tensor.transpose(pt[:], f1a[0:C, j * 128:(j + 1) * 128], ident[0:C, 0:C])
                    nc.scalar.copy(fT1[:, j * 65:j * 65 + C], pt[:])
            attn_A_chunk(st1, 7)

            def fill_chunks(st):
                return [(lambda st=st, k=k: attn_A_chunk(st, k))
                        for k in range(len(st["achunks"]))]

            emit_pools2()
            st2 = attn_A_start(f1a, N1, 2)
            attn_finish(st0, fT1, w1, filler=fill_chunks(st2))
            st3 = attn_A_start(f1a, N1, 3)
            attn_finish(st1, fT1, w1, filler=fill_chunks(st3))
            fT2 = build_fT(f2a, 8, "fT2")
            st4 = attn_A_start(f1a, N1, 4)
            attn_finish(st2, fT1, w1, filler=fill_chunks(st4))
            attn_sb(f2a, fT2, 1024, 0, w2)
            st5 = attn_A_start(f1a, N1, 5)
            attn_finish(st3, fT1, w1, filler=fill_chunks(st5))
            attn_sb(f2a, fT2, 1024, 1, w2)
            emit_pools4()
            st6 = attn_A_start(f1a, N1, 6)
            attn_finish(st4, fT1, w1, filler=fill_chunks(st6))
            fT4 = build_fT(f4a, 2, "fT4")
            attn_sb(f4a, fT4, 256, 0, w4)
            emit_up4()
            st7 = attn_A_start(f1a, N1, 7)
            attn_finish(st5, fT1, w1, filler=fill_chunks(st7))
            emit_up2()
            attn_finish(st6, fT1, w1)
            # last superblock: the final up_acc add + most of the output DMA
            # overlap its B/C window (DVE/DMA are otherwise idle there).
            nc.vector.tensor_tensor(out_acc[:, 0:3584], out_acc[:, 0:3584],
                                    up_acc[:, 0:3584], op=ALU.add)
            nc.sync.dma_start(out_d.ap()[:, 0:3584], out_acc[:, 0:3584])
            attn_finish(st7, fT1, w1)
            nc.gpsimd.tensor_tensor(out_acc[:, 3584:N1], out_acc[:, 3584:N1],
                                    up_acc[:, 3584:N1], op=ALU.add)
            nc.sync.dma_start(out_d.ap()[:, 3584:N1], out_acc[:, 3584:N1])

    nc.compile()
    return nc


def _prep_inputs(x, W_std):
    lap = np.array([[0., 1., 0.], [1., -4., 1.], [0., 1., 0.]], dtype=np.float32)
    Wl = W_std.astype(np.float32) + lap[None, None] * np.eye(C, dtype=np.float32)[:, :, None, None]
    wt = np.ascontiguousarray(Wl.transpose(1, 2, 3, 0).reshape(C, 9 * C)).astype(np.float16)
    B = x.shape[0]
    xps = np.zeros((B, C, PAD, PAD), dtype=np.float16)
    xps[:, :, 1:H + 1, 1:W + 1] = x.astype(np.float16)
    return xps.reshape(B, C, PAD * PAD), wt


def _run(x, W_std, trace=False):
    x = np.asarray(x)
    W_std = np.asarray(W_std)
    xps, wt = _prep_inputs(x, W_std)
    if "nc" not in _cache:
        _cache["nc"] = _build_nc()
    nc = _cache["nc"]
    in_maps = [{"xp": np.ascontiguousarray(xps[i]), "wt": wt} for i in range(x.shape[0])]
    ncores = min(NCORES, x.shape[0])
    res = run_bass_kernel_spmd(nc, in_maps, core_ids=list(range(ncores)), trace=trace)
    out = np.stack([res.results[i]["out"].reshape(C, H, W) for i in range(x.shape[0])])
    return out.astype(np.float32), res


def kernel(x, W_std):
    out, _ = _run(x, W_std, trace=False)
    return out
